# revision 1
# baseline (speedup 1.0000x reference)
"""4D multilinear interpolation (8x8x8x8 lattice) on 8 Trainium2 cores.

For each row b: scale coordinates[b] (4 values in [0,1)) to the 7-cell
lattice, find the containing cell, gather the 16 corner values from
mesh_pred[b] (4096 values), and blend with multilinear weights.

HW constraint (measured): indirect DMA gather consumes ONE index per
partition and streams the dest free-width contiguously from it.  So rows
are laid out b = n*128 + p (host pre-permutes coordinates into (p,n)
order; output is permuted back) and each of the 32 gathers fetches, per
partition, the 586-float span that covers all 16 cell corners of one row.
Corner extraction is then a fixed multi-dim strided view ([512,2],[64,2],
[8,2],[1,2]) of the gathered span; the weighted blend runs as a handful
of wide DVE ops instead of per-row arithmetic.
"""

import numpy as np

import concourse.bass as bass
import concourse.bacc as bacc
import concourse.mybir as mybir
from concourse import bass_utils
from concourse.tile import TileContext

P = 128          # partitions
I = 32           # row-tiles (gathers) per core
BC = P * I       # 4096 rows per core
VOL = 4096       # 8^4 lattice values per row
ND = 4
NCORES = 8
MESH = 8
SPANW = 640      # padded per-row gather width (586 used)
SPAN = 586       # 585 max corner offset + 1
F32 = mybir.dt.float32
I32 = mybir.dt.int32
OP = mybir.AluOpType


def _build():
    nc = bacc.Bacc("TRN2", target_bir_lowering=False, debug=False)
    # coordinates arrive host-permuted: device row p*I+n = original row n*P+p
    coords = nc.dram_tensor("coordinates", [BC, ND], F32, kind="ExternalInput")
    mesh = nc.dram_tensor("mesh_pred", [BC, VOL], F32, kind="ExternalInput")
    out = nc.dram_tensor("out", [BC], F32, kind="ExternalOutput")

    mesh_2d = mesh[:]
    coords_t = coords[:].rearrange("(p n) d -> p (n d)", p=P)
    out_t = out[:].rearrange("(p n) -> p n", p=P)  # host permutes back

    with TileContext(nc) as tc:
        with tc.tile_pool(name="pool", bufs=1) as pool:
            ct = pool.tile([P, I * ND], F32, tag="ct")
            nc.sync.dma_start(out=ct[:], in_=coords_t)

            # flat row base for original row n*P+p: (n*P+p)*VOL
            # iota pattern steps are int16-limited, so generate n*P+p and
            # shift left by log2(VOL) on DVE (also absorbs the Pool sem)
            tbl = pool.tile([P, I], I32, tag="tbl")
            nc.gpsimd.iota(tbl[:], pattern=[[P, I]], base=0, channel_multiplier=1)
            c = pool.tile([P, I * ND], F32, tag="c")
            nc.vector.tensor_scalar_mul(c[:], ct[:], float(MESH - 1))
            tbl2 = pool.tile([P, I], I32, tag="tbl2")
            nc.vector.tensor_scalar(
                out=tbl2[:], in0=tbl[:], scalar1=12, scalar2=None,
                op0=OP.logical_shift_left,
            )

            # floor via comparison sums (independent, then shallow tree)
            ges = []
            for k in range(1, MESH - 1):
                g = pool.tile([P, I * ND], F32, tag=f"ge{k}")
                nc.vector.tensor_scalar(
                    out=g[:], in0=c[:], scalar1=float(k), scalar2=None, op0=OP.is_ge
                )
                ges.append(g)
            while len(ges) > 1:
                nxt = []
                for a in range(0, len(ges) - 1, 2):
                    s = pool.tile([P, I * ND], F32, tag=f"gs{len(ges)}_{a}")
                    nc.vector.tensor_tensor(
                        out=s[:], in0=ges[a][:], in1=ges[a + 1][:], op=OP.add
                    )
                    nxt.append(s)
                if len(ges) % 2:
                    nxt.append(ges[-1])
                ges = nxt
            cif = ges[0]

            frac = pool.tile([P, I * ND], F32, tag="frac")
            nc.vector.tensor_tensor(out=frac[:], in0=c[:], in1=cif[:], op=OP.subtract)

            # lattice idx = sum_d cif_d * coef_d (exact in f32)
            cc = pool.tile([P, I * ND], F32, tag="cc")
            for d, coef in enumerate((512.0, 64.0, 8.0, 1.0)):
                nc.vector.tensor_scalar_mul(
                    cc[:, d * I:(d + 1) * I], cif[:, d::ND], coef
                )
            s1a = pool.tile([P, I], F32, tag="s1a")
            s1b = pool.tile([P, I], F32, tag="s1b")
            nc.vector.tensor_tensor(out=s1a[:], in0=cc[:, 0:I], in1=cc[:, I:2 * I], op=OP.add)
            nc.vector.tensor_tensor(out=s1b[:], in0=cc[:, 2 * I:3 * I], in1=cc[:, 3 * I:], op=OP.add)
            idxf = pool.tile([P, I], F32, tag="idxf")
            nc.vector.tensor_tensor(out=idxf[:], in0=s1a[:], in1=s1b[:], op=OP.add)
            idxi = pool.tile([P, I], I32, tag="idxi")
            nc.vector.tensor_copy(out=idxi[:], in_=idxf[:])
            idx = pool.tile([P, I], I32, tag="idx")
            nc.vector.tensor_tensor(out=idx[:], in0=idxi[:], in1=tbl2[:], op=OP.add)

            # 32 per-partition span gathers into one contiguous buffer
            Gbig = pool.tile([P, I * SPANW], F32, tag="Gbig")
            for n in range(I):
                nc.gpsimd.indirect_dma_start(
                    out=Gbig[:, n * SPANW:n * SPANW + SPAN],
                    out_offset=None,
                    in_=mesh_2d,
                    in_offset=bass.IndirectOffsetOnAxis(ap=idx[:, n:n + 1], axis=1),
                    element_offset=0,
                )

            # weights: om=1-frac; w01[(g,n)], w23[(j,n)]; W16[(n,k)] k=(a,b,c,d)
            om = pool.tile([P, I * ND], F32, tag="om")
            nc.vector.tensor_scalar(
                out=om[:], in0=frac[:], scalar1=-1.0, scalar2=1.0,
                op0=OP.mult, op1=OP.add,
            )
            w01 = pool.tile([P, 4 * I], F32, tag="w01")
            w23 = pool.tile([P, 4 * I], F32, tag="w23")
            pairs = ((0, 0), (0, 1), (1, 0), (1, 1))
            for g, (a, b) in enumerate(pairs):
                nc.vector.tensor_tensor(
                    out=w23[:, g * I:(g + 1) * I],
                    in0=(frac if a else om)[:, 2::ND],
                    in1=(frac if b else om)[:, 3::ND], op=OP.mult,
                )
            for g, (a, b) in enumerate(pairs):
                nc.vector.tensor_tensor(
                    out=w01[:, g * I:(g + 1) * I],
                    in0=(frac if a else om)[:, 0::ND],
                    in1=(frac if b else om)[:, 1::ND], op=OP.mult,
                )
            W16 = pool.tile([P, I * 16], F32, tag="W16")  # layout (n, k) k fastest
            for k in range(16):
                g, j = k >> 2, k & 3
                nc.vector.tensor_tensor(
                    out=W16[:, k::16],
                    in0=w01[:, g * I:(g + 1) * I],
                    in1=w23[:, j * I:(j + 1) * I], op=OP.mult,
                )

            # fused blend: per (a,b) corner-pair plane, wide mult of the
            # strided corner view against the matching W16 view
            W16v = W16[:].rearrange("p (n k) -> p n k", k=16)
            M = []
            for ab in range(4):
                a, b = ab >> 1, ab & 1
                goff = a * 512 + b * 64
                # corner view: [p][n: SPANW][c: 8, 2][d: 1, 2] at offset goff
                gview = Gbig[:]
                gview = bass.AP(
                    gview.tensor,
                    gview.offset + goff,
                    [gview.ap[0], [SPANW, I], [8, 2], [1, 2]],
                )
                wview = bass.AP(
                    W16v.tensor,
                    W16v.offset + ab * 4,
                    [W16v.ap[0], W16v.ap[1], [2, 2], [1, 2]],
                )
                m = pool.tile([P, I * 4], F32, tag=f"M{ab}")
                nc.vector.tensor_tensor(
                    out=m[:].rearrange("p (n c d) -> p n c d", c=2, d=2),
                    in0=gview, in1=wview, op=OP.mult,
                )
                M.append(m)
            m01 = pool.tile([P, I * 4], F32, tag="m01")
            m23 = pool.tile([P, I * 4], F32, tag="m23")
            msum = pool.tile([P, I * 4], F32, tag="msum")
            nc.vector.tensor_tensor(out=m01[:], in0=M[0][:], in1=M[1][:], op=OP.add)
            nc.vector.tensor_tensor(out=m23[:], in0=M[2][:], in1=M[3][:], op=OP.add)
            nc.vector.tensor_tensor(out=msum[:], in0=m01[:], in1=m23[:], op=OP.add)
            # reduce (c,d): adjacent pairs twice
            t1 = pool.tile([P, I * 2], F32, tag="t1")
            nc.vector.tensor_tensor(
                out=t1[:], in0=msum[:, 0::2], in1=msum[:, 1::2], op=OP.add
            )
            acc = pool.tile([P, I], F32, tag="acc")
            nc.vector.tensor_tensor(
                out=acc[:], in0=t1[:, 0::2], in1=t1[:, 1::2], op=OP.add
            )

            nc.sync.dma_start(out=out_t, in_=acc[:])
    nc.compile()
    return nc


_NC = None


def _get_nc():
    global _NC
    if _NC is None:
        _NC = _build()
    return _NC


def kernel(coordinates, mesh_pred, _trace=False, _tmpdir=None):
    coordinates = np.asarray(coordinates, dtype=np.float32)
    mesh_pred = np.asarray(mesh_pred, dtype=np.float32)
    assert coordinates.shape == (NCORES * BC, ND)
    assert mesh_pred.shape == (NCORES * BC, VOL)

    in_maps = []
    for cix in range(NCORES):
        sl = slice(cix * BC, (cix + 1) * BC)
        cs = coordinates[sl]
        # device row p*I+n must hold original row n*P+p
        cs_perm = np.ascontiguousarray(
            cs.reshape(I, P, ND).transpose(1, 0, 2).reshape(BC, ND)
        )
        in_maps.append(
            {
                "coordinates": cs_perm,
                "mesh_pred": np.ascontiguousarray(mesh_pred[sl]),
            }
        )
    res = bass_utils.run_bass_kernel_spmd(
        _get_nc(),
        in_maps,
        core_ids=list(range(NCORES)),
        trace=_trace,
        tmpdir=_tmpdir,
    )
    outs = []
    for r in res.results:
        o = np.asarray(r["out"]).reshape(P, I)  # [p, n]
        outs.append(o.transpose(1, 0).reshape(-1))  # back to b = n*P+p
    out = np.concatenate(outs)
    if _trace:
        return out, res
    return out



# revision 4
# speedup vs baseline: 1.0056x; 1.0056x over previous
"""4D multilinear interpolation (8x8x8x8 lattice) on 8 Trainium2 cores.

For each row b: scale coordinates[b] (4 values in [0,1)) to the 7-cell
lattice, find the containing cell, gather the 16 corner values from
mesh_pred[b] (4096 values), and blend with multilinear weights.

HW constraint (measured): indirect DMA gather consumes ONE index per
partition and streams the dest free-width contiguously from it.  So rows
are laid out b = n*128 + p (host pre-permutes coordinates into (p,n)
order; output is permuted back) and each of the 32 gathers fetches, per
partition, the 586-float span that covers all 16 cell corners of one row.
Corner extraction is then a fixed multi-dim strided view ([512,2],[64,2],
[8,2],[1,2]) of the gathered span; the weighted blend runs as a handful
of wide DVE ops instead of per-row arithmetic.
"""

import numpy as np

import concourse.bass as bass
import concourse.bacc as bacc
import concourse.mybir as mybir
from concourse import bass_utils
from concourse.tile import TileContext

P = 128          # partitions
I = 32           # row-tiles (gathers) per core
BC = P * I       # 4096 rows per core
VOL = 4096       # 8^4 lattice values per row
ND = 4
NCORES = 8
MESH = 8
SPANW = 640      # padded per-row gather width (586 used)
SPAN = 586       # 585 max corner offset + 1
F32 = mybir.dt.float32
I32 = mybir.dt.int32
OP = mybir.AluOpType


def _build():
    nc = bacc.Bacc("TRN2", target_bir_lowering=False, debug=False)
    # coordinates arrive host-permuted: device row p*I+n = original row n*P+p
    coords = nc.dram_tensor("coordinates", [BC, ND], F32, kind="ExternalInput")
    mesh = nc.dram_tensor("mesh_pred", [BC, VOL], F32, kind="ExternalInput")
    out = nc.dram_tensor("out", [BC], F32, kind="ExternalOutput")

    mesh_2d = mesh[:]
    coords_t = coords[:].rearrange("(p n) d -> p (n d)", p=P)
    out_t = out[:].rearrange("(p n) -> p n", p=P)  # host permutes back

    with TileContext(nc) as tc:
        with tc.tile_pool(name="pool", bufs=1) as pool:
            ct = pool.tile([P, I * ND], F32, tag="ct")
            nc.sync.dma_start(out=ct[:], in_=coords_t)

            # flat row base for original row n*P+p: (n*P+p)*VOL
            # iota pattern steps are int16-limited, so generate n*P+p and
            # shift left by log2(VOL) on DVE (also absorbs the Pool sem)
            tbl = pool.tile([P, I], I32, tag="tbl")
            nc.gpsimd.iota(tbl[:], pattern=[[P, I]], base=0, channel_multiplier=1)
            c = pool.tile([P, I * ND], F32, tag="c")
            nc.vector.tensor_scalar_mul(c[:], ct[:], float(MESH - 1))
            tbl2 = pool.tile([P, I], I32, tag="tbl2")
            nc.vector.tensor_scalar(
                out=tbl2[:], in0=tbl[:], scalar1=12, scalar2=None,
                op0=OP.logical_shift_left,
            )

            # floor via comparison sums (independent, then shallow tree)
            ges = []
            for k in range(1, MESH - 1):
                g = pool.tile([P, I * ND], F32, tag=f"ge{k}")
                nc.vector.tensor_scalar(
                    out=g[:], in0=c[:], scalar1=float(k), scalar2=None, op0=OP.is_ge
                )
                ges.append(g)
            while len(ges) > 1:
                nxt = []
                for a in range(0, len(ges) - 1, 2):
                    s = pool.tile([P, I * ND], F32, tag=f"gs{len(ges)}_{a}")
                    nc.vector.tensor_tensor(
                        out=s[:], in0=ges[a][:], in1=ges[a + 1][:], op=OP.add
                    )
                    nxt.append(s)
                if len(ges) % 2:
                    nxt.append(ges[-1])
                ges = nxt
            cif = ges[0]

            frac = pool.tile([P, I * ND], F32, tag="frac")
            nc.vector.tensor_tensor(out=frac[:], in0=c[:], in1=cif[:], op=OP.subtract)

            # lattice idx = sum_d cif_d * coef_d (exact in f32)
            cc = pool.tile([P, I * ND], F32, tag="cc")
            for d, coef in enumerate((512.0, 64.0, 8.0, 1.0)):
                nc.vector.tensor_scalar_mul(
                    cc[:, d * I:(d + 1) * I], cif[:, d::ND], coef
                )
            s1a = pool.tile([P, I], F32, tag="s1a")
            s1b = pool.tile([P, I], F32, tag="s1b")
            nc.vector.tensor_tensor(out=s1a[:], in0=cc[:, 0:I], in1=cc[:, I:2 * I], op=OP.add)
            nc.vector.tensor_tensor(out=s1b[:], in0=cc[:, 2 * I:3 * I], in1=cc[:, 3 * I:], op=OP.add)
            idxf = pool.tile([P, I], F32, tag="idxf")
            nc.vector.tensor_tensor(out=idxf[:], in0=s1a[:], in1=s1b[:], op=OP.add)
            idxi = pool.tile([P, I], I32, tag="idxi")
            nc.vector.tensor_copy(out=idxi[:], in_=idxf[:])
            idx = pool.tile([P, I], I32, tag="idx")
            nc.vector.tensor_tensor(out=idx[:], in0=idxi[:], in1=tbl2[:], op=OP.add)

            # 32 per-partition span gathers into one contiguous buffer
            Gbig = pool.tile([P, I * SPANW], F32, tag="Gbig")
            for n in range(I):
                nc.gpsimd.indirect_dma_start(
                    out=Gbig[:, n * SPANW:n * SPANW + SPAN],
                    out_offset=None,
                    in_=mesh_2d,
                    in_offset=bass.IndirectOffsetOnAxis(ap=idx[:, n:n + 1], axis=1),
                    element_offset=0,
                )

            # weights: om=1-frac; w01[(g,n)], w23[(j,n)]; W16[(n,k)] k=(a,b,c,d)
            om = pool.tile([P, I * ND], F32, tag="om")
            nc.vector.tensor_scalar(
                out=om[:], in0=frac[:], scalar1=-1.0, scalar2=1.0,
                op0=OP.mult, op1=OP.add,
            )
            w01 = pool.tile([P, 4 * I], F32, tag="w01")
            w23 = pool.tile([P, 4 * I], F32, tag="w23")
            pairs = ((0, 0), (0, 1), (1, 0), (1, 1))
            for g, (a, b) in enumerate(pairs):
                nc.vector.tensor_tensor(
                    out=w23[:, g * I:(g + 1) * I],
                    in0=(frac if a else om)[:, 2::ND],
                    in1=(frac if b else om)[:, 3::ND], op=OP.mult,
                )
            for g, (a, b) in enumerate(pairs):
                nc.vector.tensor_tensor(
                    out=w01[:, g * I:(g + 1) * I],
                    in0=(frac if a else om)[:, 0::ND],
                    in1=(frac if b else om)[:, 1::ND], op=OP.mult,
                )
            W16 = pool.tile([P, I * 16], F32, tag="W16")  # layout (n, k) k fastest
            for k in range(16):
                g, j = k >> 2, k & 3
                nc.vector.tensor_tensor(
                    out=W16[:, k::16],
                    in0=w01[:, g * I:(g + 1) * I],
                    in1=w23[:, j * I:(j + 1) * I], op=OP.mult,
                )

            # fused blend: per (a,b) corner-pair plane, wide mult of the
            # strided corner view against the matching W16 view
            W16v = W16[:].rearrange("p (n k) -> p n k", k=16)
            M = []
            for ab in range(4):
                a, b = ab >> 1, ab & 1
                goff = a * 512 + b * 64
                # corner view: [p][n: SPANW][c: 8, 2][d: 1, 2] at offset goff
                gview = Gbig[:]
                gview = bass.AP(
                    gview.tensor,
                    gview.offset + goff,
                    [gview.ap[0], [SPANW, I], [8, 2], [1, 2]],
                )
                wview = bass.AP(
                    W16v.tensor,
                    W16v.offset + ab * 4,
                    [W16v.ap[0], W16v.ap[1], [2, 2], [1, 2]],
                )
                m = pool.tile([P, I * 4], F32, tag=f"M{ab}")
                nc.vector.tensor_tensor(
                    out=m[:].rearrange("p (n c d) -> p n c d", c=2, d=2),
                    in0=gview, in1=wview, op=OP.mult,
                )
                M.append(m)
            m01 = pool.tile([P, I * 4], F32, tag="m01")
            m23 = pool.tile([P, I * 4], F32, tag="m23")
            msum = pool.tile([P, I * 4], F32, tag="msum")
            nc.vector.tensor_tensor(out=m01[:], in0=M[0][:], in1=M[1][:], op=OP.add)
            nc.vector.tensor_tensor(out=m23[:], in0=M[2][:], in1=M[3][:], op=OP.add)
            nc.vector.tensor_tensor(out=msum[:], in0=m01[:], in1=m23[:], op=OP.add)
            # reduce (c,d): adjacent pairs twice
            t1 = pool.tile([P, I * 2], F32, tag="t1")
            nc.vector.tensor_tensor(
                out=t1[:], in0=msum[:, 0::2], in1=msum[:, 1::2], op=OP.add
            )
            acc = pool.tile([P, I], F32, tag="acc")
            nc.vector.tensor_tensor(
                out=acc[:], in0=t1[:, 0::2], in1=t1[:, 1::2], op=OP.add
            )

            nc.sync.dma_start(out=out_t, in_=acc[:])
    nc.compile()
    return nc


_NC = None


def _get_nc():
    global _NC
    if _NC is None:
        _NC = _build()
    return _NC


def kernel(coordinates, mesh_pred, _trace=False, _tmpdir=None):
    coordinates = np.asarray(coordinates, dtype=np.float32)
    mesh_pred = np.asarray(mesh_pred, dtype=np.float32)
    assert coordinates.shape == (NCORES * BC, ND)
    assert mesh_pred.shape == (NCORES * BC, VOL)

    in_maps = []
    for cix in range(NCORES):
        sl = slice(cix * BC, (cix + 1) * BC)
        cs = coordinates[sl]
        # device row p*I+n must hold original row n*P+p
        cs_perm = np.ascontiguousarray(
            cs.reshape(I, P, ND).transpose(1, 0, 2).reshape(BC, ND)
        )
        in_maps.append(
            {
                "coordinates": cs_perm,
                "mesh_pred": np.ascontiguousarray(mesh_pred[sl]),
            }
        )
    res = bass_utils.run_bass_kernel_spmd(
        _get_nc(),
        in_maps,
        core_ids=list(range(NCORES)),
        trace=_trace,
        tmpdir=_tmpdir,
    )
    outs = []
    for r in res.results:
        o = np.asarray(r["out"]).reshape(P, I)  # [p, n]
        outs.append(o.transpose(1, 0).reshape(-1))  # back to b = n*P+p
    out = np.concatenate(outs)
    if _trace:
        return out, res
    return out

